# revision 3
# baseline (speedup 1.0000x reference)
"""Trainium2 Bass kernel for nn_Density_Block (histogram_binning).

Computes, for N=1M rows:
    out1       = softmax(x @ weight + bias, axis=1)        [N, 101]
    out_interp = lerp of two adjacent bins of out1 at t*B   [N]

Strategy (8 NeuronCores, pure data parallel):
  * Host sorts rows by lower-bin index Li and interleaves sorted rows
    round-robin across cores (core i gets sorted rows i::8).  This makes
    the per-row dual-gather STATIC: every 128-row device tile j (on every
    core) draws its two softmax bins from a fixed 4-wide column window
    LO[j] known at program-build time.
  * Host passes x pre-transposed with a ones-row appended ([65, R] per
    core) so the matmul adds the bias for free, plus a per-row 4-wide
    coefficient vector encoding the interpolation weights.
  * Device per 128-row tile: fp32 matmul (K=65, N=102) -> PSUM; ScalarE
    exp with fused accumulate (softmax denominator); VectorE reciprocal +
    per-partition scalar multiply (normalize); fused tensor_tensor_reduce
    dot of the 4-wide window with the coefficients -> out_interp.
  * Bins padded 101 -> 102 (pad logit = -100 => exp ~ 0) to keep the DVE
    innermost dim even (2x perf mode) and PSUM groups bank-aligned
    (5 tiles x 102 = 510 <= 512 floats = 1 PSUM bank).
"""

import os
import sys
import types
from contextlib import ExitStack

import numpy as np

import concourse.bass as bass
import concourse.tile as tile
from concourse import bacc, mybir
from concourse import bass_utils

F32 = mybir.dt.float32

N_CORES = 8
NBINS = 101
NB_PAD = 102
IND = 64
K = IND + 1          # features + ones row (bias)
TILE = 128           # rows per tile (SBUF partitions)
TPG = 5              # tiles per group (5 * 102 = 510 floats = one PSUM bank)
GROUP_ROWS = TILE * TPG
WIN = 4              # gather window width (needs intra-window Li spread <= 2)
PAD_LOGIT = -100.0   # pad-bin logit: exp(-100) ~ 0

# Stashed by kernel() for the local test harness (ignored by grading).
LAST_RESULT = None


def _install_ntff_hook():
    """Register the axon NTFF profiling hook if the image lacks antenv.axon_hooks."""
    try:
        from antenv.axon_hooks import get_axon_ntff_profile_hook  # noqa: F401
        return
    except ImportError:
        pass
    try:
        import antenv
        from trn_agent_boot.trn_boot import _ntff_profile_via_ctypes
        mod = types.ModuleType("antenv.axon_hooks")
        hook = [_ntff_profile_via_ctypes("/opt/axon/libaxon_pjrt.so")]
        mod.set_axon_ntff_profile_hook = lambda h: hook.__setitem__(0, h)
        mod.get_axon_ntff_profile_hook = lambda: hook[0]
        sys.modules["antenv.axon_hooks"] = mod
        antenv.axon_hooks = mod
    except Exception:
        pass


def host_prepare(t, x, weight, bias, num_grid, n_cores=N_CORES):
    """Sort/shard/precompute on host.  Returns (meta, per-core input maps)."""
    t = np.ascontiguousarray(np.asarray(t, dtype=np.float32))
    x = np.asarray(x, dtype=np.float32)
    weight = np.asarray(weight, dtype=np.float32)
    bias = np.asarray(bias, dtype=np.float32)
    B = int(num_grid)
    N = t.shape[0]

    # Bin indices / interpolation weight, float32-exact vs the reference.
    tB = t * np.float32(B)
    U = np.ceil(tB)
    inter = np.float32(1.0) - (U - tB)
    L = U - np.float32(1.0)
    L = np.where(L < 0, L + np.float32(1.0), L)
    Li = L.astype(np.int32)
    Ui = U.astype(np.int32)

    # Global padded size: multiple of n_cores * GROUP_ROWS.
    chunk = n_cores * GROUP_ROWS
    NP = ((N + chunk - 1) // chunk) * chunk
    npad = NP - N
    R = NP // n_cores                 # rows per core
    n_tiles = R // TILE               # local tiles per core == global windows
    winrows = TILE * n_cores          # rows per global window

    perm = np.argsort(Li, kind="stable")
    Li_s = np.concatenate([Li[perm], np.full(npad, Li[perm[-1]] if N else 0, np.int32)])
    Ui_s = np.concatenate([Ui[perm], np.full(npad, 0, np.int32)])
    inter_s = np.concatenate([inter[perm], np.zeros(npad, np.float32)])

    # Window low column per tile index (shared across cores by construction).
    LO = np.minimum(Li_s[::winrows], NB_PAD - WIN).astype(np.int32)
    assert LO.shape[0] == n_tiles
    # Validity: every row's Li and Ui must fall in [LO, LO+WIN).
    lo_per_row = np.repeat(LO, winrows)
    spread_ok = (Li_s - lo_per_row >= 0) & (np.maximum(Li_s, Ui_s) - lo_per_row < WIN)
    if not spread_ok[:N].all():
        bad = np.flatnonzero(~spread_ok[:N])[:5]
        raise AssertionError(f"gather-window assumption violated at sorted rows {bad}")

    # Per-row 4-wide coefficients (pads stay zero => out_interp 0 there).
    coef_s = np.zeros((NP, WIN), np.float32)
    rows = np.arange(N)
    np.add.at(coef_s, (rows, (Li_s[:N] - lo_per_row[:N])), np.float32(1.0) - inter_s[:N])
    np.add.at(coef_s, (rows, (Ui_s[:N] - lo_per_row[:N])), inter_s[:N])

    # Weight+bias, padded bins.
    wb = np.zeros((K, NB_PAD), np.float32)
    wb[:IND, :NBINS] = weight
    wb[IND, :NBINS] = bias
    wb[IND, NBINS] = np.float32(PAD_LOGIT)

    # Sorted+padded x, transposed with ones row, sharded row-interleaved.
    xs = np.zeros((NP, IND), np.float32)
    xs[:N] = x[perm]
    in_maps = []
    for i in range(n_cores):
        xi = np.empty((K, R), np.float32)
        xi[:IND] = xs[i::n_cores].T
        xi[IND] = 1.0
        in_maps.append({
            "xT": xi,
            "wb": wb,
            "coef": np.ascontiguousarray(coef_s[i::n_cores]),
        })

    meta = dict(N=N, NP=NP, R=R, n_tiles=n_tiles, LO=LO, perm=perm, n_cores=n_cores)
    return meta, in_maps


def build_program(LO, R, n_cores=N_CORES):
    """Build + compile the (SPMD-identical) Bass program for one core."""
    n_tiles = R // TILE
    n_groups = R // GROUP_ROWS
    assert n_groups * GROUP_ROWS == R and len(LO) == n_tiles

    nc = bacc.Bacc("TRN2", target_bir_lowering=False, debug=False,
                   num_devices=n_cores)
    xT = nc.dram_tensor("xT", [K, R], F32, kind="ExternalInput").ap()
    wb = nc.dram_tensor("wb", [K, NB_PAD], F32, kind="ExternalInput").ap()
    coef = nc.dram_tensor("coef", [R, WIN], F32, kind="ExternalInput").ap()
    out1 = nc.dram_tensor("out1", [R, NBINS], F32, kind="ExternalOutput").ap()
    oint = nc.dram_tensor("oint", [R, 1], F32, kind="ExternalOutput").ap()

    Exp = mybir.ActivationFunctionType.Exp
    mult = mybir.AluOpType.mult
    add = mybir.AluOpType.add

    with tile.TileContext(nc) as tc:
        with ExitStack() as ctx:
            wpool = ctx.enter_context(tc.tile_pool(name="w", bufs=1))
            xpool = ctx.enter_context(tc.tile_pool(name="x", bufs=3))
            cpool = ctx.enter_context(tc.tile_pool(name="c", bufs=3))
            ppool = ctx.enter_context(tc.tile_pool(name="ps", bufs=4, space="PSUM"))
            epool = ctx.enter_context(tc.tile_pool(name="ex", bufs=3))
            opool = ctx.enter_context(tc.tile_pool(name="o1", bufs=3))
            spool = ctx.enter_context(tc.tile_pool(name="sm", bufs=4))
            tpool = ctx.enter_context(tc.tile_pool(name="tt", bufs=3))

            wbt = wpool.tile([K, NB_PAD], F32)
            nc.sync.dma_start(wbt[:], wb[:])

            for g in range(n_groups):
                r0 = g * GROUP_ROWS
                xin = xpool.tile([K, GROUP_ROWS], F32)
                nc.sync.dma_start(xin[:], xT[:, r0:r0 + GROUP_ROWS])
                cf = cpool.tile([128, TPG * WIN], F32)
                nc.sync.dma_start(
                    cf[:].rearrange("p (t c) -> p t c", t=TPG),
                    coef[r0:r0 + GROUP_ROWS, :].rearrange("(t p) c -> p t c", p=128),
                )

                ps = ppool.tile([128, TPG * NB_PAD], F32)
                for ti in range(TPG):
                    nc.tensor.matmul(
                        ps[:, ti * NB_PAD:(ti + 1) * NB_PAD],
                        lhsT=xin[:, ti * TILE:(ti + 1) * TILE],
                        rhs=wbt[:],
                        start=True, stop=True,
                    )

                ex = epool.tile([128, TPG * NB_PAD], F32)
                sg = spool.tile([128, TPG], F32)
                for ti in range(TPG):
                    nc.scalar.activation(
                        ex[:, ti * NB_PAD:(ti + 1) * NB_PAD],
                        ps[:, ti * NB_PAD:(ti + 1) * NB_PAD],
                        Exp, accum_out=sg[:, ti:ti + 1],
                    )

                rg = spool.tile([128, TPG], F32)
                nc.vector.reciprocal(rg[:], sg[:])

                o1 = opool.tile([128, TPG * NB_PAD], F32)
                for ti in range(TPG):
                    nc.vector.tensor_scalar_mul(
                        o1[:, ti * NB_PAD:(ti + 1) * NB_PAD],
                        ex[:, ti * NB_PAD:(ti + 1) * NB_PAD],
                        rg[:, ti:ti + 1],
                    )

                # out_interp tile = sum((exp_window * 1/s) * coef) via the
                # fused scalar_tensor_tensor accumulate (TTR is not supported
                # by this runtime's exec units).
                tt = tpool.tile([128, TPG * WIN], F32)
                oi = spool.tile([128, TPG], F32)
                for ti in range(TPG):
                    lo = int(LO[g * TPG + ti])
                    nc.vector.scalar_tensor_tensor(
                        out=tt[:, ti * WIN:(ti + 1) * WIN],
                        in0=ex[:, ti * NB_PAD + lo: ti * NB_PAD + lo + WIN],
                        scalar=rg[:, ti:ti + 1],
                        in1=cf[:, ti * WIN:(ti + 1) * WIN],
                        op0=mult, op1=mult,
                        accum_out=oi[:, ti:ti + 1],
                    )

                nc.sync.dma_start(
                    out1[r0:r0 + GROUP_ROWS, :].rearrange("(t p) c -> p t c", p=128),
                    o1[:].rearrange("p (t c) -> p t c", t=TPG)[:, :, 0:NBINS],
                )
                nc.sync.dma_start(
                    oint[r0:r0 + GROUP_ROWS, 0].rearrange("(t p) -> p t", p=128),
                    oi[:],
                )

    nc.compile()
    return nc


def kernel(t, x, weight, bias, num_grid):
    global LAST_RESULT
    trace = bool(os.environ.get("BASS_TRACE"))
    if trace:
        _install_ntff_hook()
        bass_utils.upload_artifacts = lambda tmpdir: "local://" + tmpdir

    meta, in_maps = host_prepare(t, x, weight, bias, num_grid)
    nc = build_program(meta["LO"], meta["R"], meta["n_cores"])

    res = bass_utils.run_bass_kernel_spmd(
        nc, in_maps, core_ids=list(range(meta["n_cores"])), trace=trace,
    )
    LAST_RESULT = res

    N, NP, n_cores = meta["N"], meta["NP"], meta["n_cores"]
    perm = meta["perm"]
    out1_s = np.empty((NP, NBINS), np.float32)
    oint_s = np.empty((NP,), np.float32)
    for i in range(n_cores):
        out1_s[i::n_cores] = res.results[i]["out1"]
        oint_s[i::n_cores] = res.results[i]["oint"][:, 0]
    out1 = np.empty((N, NBINS), np.float32)
    oint = np.empty((N,), np.float32)
    out1[perm] = out1_s[:N]
    oint[perm] = oint_s[:N]
    return out1, oint


# revision 6
# speedup vs baseline: 2.7721x; 2.7721x over previous
"""Trainium2 Bass kernel for nn_Density_Block (histogram_binning).

Computes, for N=1M rows:
    out1       = softmax(x @ weight + bias, axis=1)        [N, 101]
    out_interp = lerp of two adjacent bins of out1 at t*B   [N]

Strategy (8 NeuronCores, pure data parallel):
  * Host sorts rows by lower-bin index Li and interleaves sorted rows
    round-robin across cores (core i gets sorted rows i::8).  This makes
    the per-row dual-gather STATIC: each group of device tiles draws its
    two softmax bins from a fixed 4-wide column window known at
    program-build time (identical across cores by construction).
  * Host passes x pre-transposed, split into bf16 hi/lo halves, with a
    ones-row appended so the matmul adds the bias for free, plus per-row
    4-wide coefficients encoding the interpolation weights.
  * Matmul runs as 3 bf16 matmuls (xh@Wh + xh@Wl + xl@Wh) accumulated in
    fp32 PSUM — ~10x faster than the fp32 PE path at ~1e-4 accuracy.
  * Per 10-tile group (1280 rows, 2 PSUM banks): grouped exp on ScalarE,
    softmax sums via segmented DVE reduce, reciprocal, normalize on the
    otherwise-idle GpSimd (broadcast multiply), interp dot via one
    multiply + segmented reduce that lands directly in the spare pad
    column of the output tile, so one combined DMA writes both outputs.
  * Bins padded 101 -> 102: pad logit = -100 (exp ~ 0) and the pad
    column doubles as the out_interp carrier.
"""

import os
import sys
import types
from contextlib import ExitStack

import numpy as np

import concourse.bass as bass
import concourse.tile as tile
from concourse import bacc, mybir
from concourse import bass_utils

F32 = mybir.dt.float32
BF16 = mybir.dt.bfloat16

N_CORES = 8
NBINS = 101
NB_PAD = 102
IND = 64
K = IND + 1          # features + ones row (bias)
TILE = 128           # rows per tile (SBUF partitions)
TPB = 5              # tiles per PSUM bank (5 * 102 = 510 <= 512 floats)
BPG = 2              # PSUM banks per group
TPG = TPB * BPG      # tiles per group = 10
GROUP_ROWS = TILE * TPG          # 1280
WINDOW_TILES = TPB               # tiles sharing one gather window
WIN = 4              # gather window width (needs intra-window Li spread <= 2)
PAD_LOGIT = -100.0   # pad-bin logit: exp(-100) ~ 0
BANK = 512           # PSUM bank stride in f32 elements

LAST_RESULT = None   # stashed for the local test harness


def _install_ntff_hook():
    try:
        from antenv.axon_hooks import get_axon_ntff_profile_hook  # noqa: F401
        return
    except ImportError:
        pass
    try:
        import antenv
        from trn_agent_boot.trn_boot import _ntff_profile_via_ctypes
        mod = types.ModuleType("antenv.axon_hooks")
        hook = [_ntff_profile_via_ctypes("/opt/axon/libaxon_pjrt.so")]
        mod.set_axon_ntff_profile_hook = lambda h: hook.__setitem__(0, h)
        mod.get_axon_ntff_profile_hook = lambda: hook[0]
        sys.modules["antenv.axon_hooks"] = mod
        antenv.axon_hooks = mod
    except Exception:
        pass


def host_prepare(t, x, weight, bias, num_grid, n_cores=N_CORES):
    """Sort/shard/precompute on host.  Returns (meta, per-core input maps)."""
    t = np.ascontiguousarray(np.asarray(t, dtype=np.float32))
    x = np.asarray(x, dtype=np.float32)
    weight = np.asarray(weight, dtype=np.float32)
    bias = np.asarray(bias, dtype=np.float32)
    B = int(num_grid)
    N = t.shape[0]

    # Bin indices / interpolation weight, float32-exact vs the reference.
    tB = t * np.float32(B)
    U = np.ceil(tB)
    inter = np.float32(1.0) - (U - tB)
    L = U - np.float32(1.0)
    L = np.where(L < 0, L + np.float32(1.0), L)
    Li = L.astype(np.int32)
    Ui = U.astype(np.int32)

    chunk = n_cores * GROUP_ROWS
    NP = ((N + chunk - 1) // chunk) * chunk
    npad = NP - N
    R = NP // n_cores                 # rows per core
    n_tiles = R // TILE
    n_windows = n_tiles // WINDOW_TILES
    winrows = TILE * n_cores * WINDOW_TILES   # global sorted rows per window

    perm = np.argsort(Li, kind="stable")
    Li_s = np.concatenate([Li[perm], np.full(npad, Li[perm[-1]] if N else 0, np.int32)])
    Ui_s = np.concatenate([Ui[perm], np.full(npad, 0, np.int32)])
    inter_s = np.concatenate([inter[perm], np.zeros(npad, np.float32)])

    # Window low column, shared by WINDOW_TILES consecutive tiles on every core.
    LO = np.minimum(Li_s[::winrows], NB_PAD - WIN).astype(np.int32)
    assert LO.shape[0] == n_windows
    lo_per_row = np.repeat(LO, winrows)
    spread_ok = (Li_s - lo_per_row >= 0) & (np.maximum(Li_s, Ui_s) - lo_per_row < WIN)
    if not spread_ok[:N].all():
        bad = np.flatnonzero(~spread_ok[:N])[:5]
        raise AssertionError(f"gather-window assumption violated at sorted rows {bad}")

    coef_s = np.zeros((NP, WIN), np.float32)
    rows = np.arange(N)
    np.add.at(coef_s, (rows, (Li_s[:N] - lo_per_row[:N])), np.float32(1.0) - inter_s[:N])
    np.add.at(coef_s, (rows, (Ui_s[:N] - lo_per_row[:N])), inter_s[:N])

    # Weight+bias, padded bins, bf16 hi/lo split.
    wb = np.zeros((K, NB_PAD), np.float32)
    wb[:IND, :NBINS] = weight
    wb[IND, :NBINS] = bias
    wb[IND, NBINS] = np.float32(PAD_LOGIT)
    import ml_dtypes
    bf16 = ml_dtypes.bfloat16
    wb_hi = wb.astype(bf16)
    wb_lo = (wb - wb_hi.astype(np.float32)).astype(bf16)

    # Sorted+padded x, transposed with ones row, sharded row-interleaved.
    xs = np.zeros((NP, IND), np.float32)
    xs[:N] = x[perm]
    in_maps = []
    for i in range(n_cores):
        xi = np.empty((K, R), np.float32)
        xi[:IND] = xs[i::n_cores].T
        xi[IND] = 1.0
        xi_hi = xi.astype(bf16)
        xi_lo = (xi - xi_hi.astype(np.float32)).astype(bf16)
        in_maps.append({
            "xh": xi_hi,
            "xl": xi_lo,
            "wh": wb_hi,
            "wl": wb_lo,
            "coef": np.ascontiguousarray(coef_s[i::n_cores]),
        })

    meta = dict(N=N, NP=NP, R=R, n_tiles=n_tiles, LO=LO, perm=perm, n_cores=n_cores)
    return meta, in_maps


def build_program(LO, R, n_cores=N_CORES):
    """Build + compile the (SPMD-identical) Bass program for one core."""
    n_tiles = R // TILE
    n_groups = R // GROUP_ROWS
    assert n_groups * GROUP_ROWS == R
    assert len(LO) == n_tiles // WINDOW_TILES

    nc = bacc.Bacc("TRN2", target_bir_lowering=False, debug=False,
                   num_devices=n_cores)
    xh = nc.dram_tensor("xh", [K, R], BF16, kind="ExternalInput").ap()
    xl = nc.dram_tensor("xl", [K, R], BF16, kind="ExternalInput").ap()
    wh = nc.dram_tensor("wh", [K, NB_PAD], BF16, kind="ExternalInput").ap()
    wl = nc.dram_tensor("wl", [K, NB_PAD], BF16, kind="ExternalInput").ap()
    coef = nc.dram_tensor("coef", [R, WIN], F32, kind="ExternalInput").ap()
    # Combined output: cols 0..100 = out1, col 101 = out_interp.
    comb = nc.dram_tensor("comb", [R, NB_PAD], F32, kind="ExternalOutput").ap()

    Exp = mybir.ActivationFunctionType.Exp
    mult = mybir.AluOpType.mult
    add = mybir.AluOpType.add
    X = mybir.AxisListType.X

    def bank_off(ti):
        return (ti // TPB) * BANK + (ti % TPB) * NB_PAD

    with tile.TileContext(nc) as tc:
        with ExitStack() as ctx:
            wpool = ctx.enter_context(tc.tile_pool(name="w", bufs=1))
            xpool = ctx.enter_context(tc.tile_pool(name="x", bufs=3))
            cpool = ctx.enter_context(tc.tile_pool(name="c", bufs=3))
            ppool = ctx.enter_context(tc.tile_pool(name="ps", bufs=3, space="PSUM"))
            epool = ctx.enter_context(tc.tile_pool(name="ex", bufs=3))
            opool = ctx.enter_context(tc.tile_pool(name="o1", bufs=3))
            spool = ctx.enter_context(tc.tile_pool(name="sm", bufs=4))
            tpool = ctx.enter_context(tc.tile_pool(name="tt", bufs=3))

            wht = wpool.tile([K, NB_PAD], BF16)
            nc.sync.dma_start(wht[:], wh[:])
            wlt = wpool.tile([K, NB_PAD], BF16)
            nc.sync.dma_start(wlt[:], wl[:])

            for g in range(n_groups):
                r0 = g * GROUP_ROWS
                xht = xpool.tile([K, GROUP_ROWS], BF16)
                nc.sync.dma_start(xht[:], xh[:, r0:r0 + GROUP_ROWS])
                xlt = xpool.tile([K, GROUP_ROWS], BF16)
                nc.sync.dma_start(xlt[:], xl[:, r0:r0 + GROUP_ROWS])
                cf = cpool.tile([128, TPG * WIN], F32)
                nc.sync.dma_start(
                    cf[:].rearrange("p (t c) -> p t c", t=TPG),
                    coef[r0:r0 + GROUP_ROWS, :].rearrange("(t p) c -> p t c", p=128),
                )

                ps = ppool.tile([128, BPG * BANK], F32)
                for ti in range(TPG):
                    o = bank_off(ti)
                    xsl = slice(ti * TILE, (ti + 1) * TILE)
                    nc.tensor.matmul(ps[:, o:o + NB_PAD], lhsT=xht[:, xsl],
                                     rhs=wht[:], start=True, stop=False)
                    nc.tensor.matmul(ps[:, o:o + NB_PAD], lhsT=xht[:, xsl],
                                     rhs=wlt[:], start=False, stop=False)
                    nc.tensor.matmul(ps[:, o:o + NB_PAD], lhsT=xlt[:, xsl],
                                     rhs=wht[:], start=False, stop=True)

                # exp, one grouped instruction per PSUM bank half
                ex = epool.tile([128, TPG * NB_PAD], F32)
                for b in range(BPG):
                    nc.scalar.activation(
                        ex[:, b * TPB * NB_PAD:(b + 1) * TPB * NB_PAD],
                        ps[:, b * BANK: b * BANK + TPB * NB_PAD],
                        Exp,
                    )

                # softmax denominators: segmented reduce [128, TPB, 102] -> [128, TPB]
                sg = spool.tile([128, TPG], F32)
                for b in range(BPG):
                    nc.vector.tensor_reduce(
                        sg[:, b * TPB:(b + 1) * TPB],
                        ex[:, b * TPB * NB_PAD:(b + 1) * TPB * NB_PAD]
                          .rearrange("p (t c) -> p t c", t=TPB),
                        axis=X, op=add,
                    )
                rg = spool.tile([128, TPG], F32)
                nc.vector.reciprocal(rg[:], sg[:])

                # normalize on GpSimd with a stride-0 broadcast of 1/s
                o1 = opool.tile([128, TPG * NB_PAD], F32)
                nc.gpsimd.tensor_tensor(
                    o1[:].rearrange("p (t c) -> p t c", t=TPG),
                    ex[:].rearrange("p (t c) -> p t c", t=TPG),
                    rg[:].broadcast_to((128, TPG, NB_PAD)),
                    op=mult,
                )

                # interp: tt = o1[window] * coef; segmented-reduce into the
                # pad column (col 101) of each tile of o1.
                tt = tpool.tile([128, TPG * WIN], F32)
                for w in range(TPG // WINDOW_TILES):
                    lo = int(LO[(g * TPG) // WINDOW_TILES + w])
                    nc.vector.tensor_tensor(
                        tt[:, w * WINDOW_TILES * WIN:(w + 1) * WINDOW_TILES * WIN]
                          .rearrange("p (t c) -> p t c", t=WINDOW_TILES),
                        o1[:].rearrange("p (t c) -> p t c", t=TPG)
                          [:, w * WINDOW_TILES:(w + 1) * WINDOW_TILES, lo:lo + WIN],
                        cf[:].rearrange("p (t c) -> p t c", t=TPG)
                          [:, w * WINDOW_TILES:(w + 1) * WINDOW_TILES, :],
                        op=mult,
                    )
                    nc.vector.tensor_reduce(
                        o1[:].rearrange("p (t c) -> p t c", t=TPG)
                          [:, w * WINDOW_TILES:(w + 1) * WINDOW_TILES, NBINS:NB_PAD],
                        tt[:, w * WINDOW_TILES * WIN:(w + 1) * WINDOW_TILES * WIN]
                          .rearrange("p (t c) -> p t c", t=WINDOW_TILES),
                        axis=X, op=add,
                    )

                nc.sync.dma_start(
                    comb[r0:r0 + GROUP_ROWS, :].rearrange("(t p) c -> p t c", p=128),
                    o1[:].rearrange("p (t c) -> p t c", t=TPG),
                )

    nc.compile()
    return nc


def kernel(t, x, weight, bias, num_grid):
    global LAST_RESULT
    trace = bool(os.environ.get("BASS_TRACE"))
    if trace:
        _install_ntff_hook()
        bass_utils.upload_artifacts = lambda tmpdir: "local://" + tmpdir

    meta, in_maps = host_prepare(t, x, weight, bias, num_grid)
    nc = build_program(meta["LO"], meta["R"], meta["n_cores"])

    res = bass_utils.run_bass_kernel_spmd(
        nc, in_maps, core_ids=list(range(meta["n_cores"])), trace=trace,
    )
    LAST_RESULT = res

    N, NP, n_cores = meta["N"], meta["NP"], meta["n_cores"]
    perm = meta["perm"]
    comb_s = np.empty((NP, NB_PAD), np.float32)
    for i in range(n_cores):
        comb_s[i::n_cores] = res.results[i]["comb"]
    out1 = np.empty((N, NBINS), np.float32)
    oint = np.empty((N,), np.float32)
    out1[perm] = comb_s[:N, :NBINS]
    oint[perm] = comb_s[:N, NBINS]
    return out1, oint


# revision 7
# speedup vs baseline: 6.4239x; 2.3174x over previous
"""Trainium2 Bass kernel for nn_Density_Block (histogram_binning).

Computes, for N=1M rows:
    out1       = softmax(x @ weight + bias, axis=1)        [N, 101]
    out_interp = lerp of two adjacent bins of out1 at t*B   [N]

Strategy (8 NeuronCores, pure data parallel):
  * Host sorts rows by lower-bin index Li and interleaves sorted rows
    round-robin across cores, so every run of 5 consecutive 128-row tiles
    (on every core) gathers its two softmax bins from one 4-wide column
    window known at program-build time.
  * The matmul x@W+b runs as TWO fp16 matmuls accumulated in fp32 PSUM:
      MM1 (K=128): [ones; x_hi; x_lo(0:63)] @ [b_hi; W_hi; W_hi(0:63)]
      MM2 (K=65):  [ones; x_hi]             @ [b_lo; W_lo]
    (x_hi/W_hi fp16, *_lo = fp16 residuals; only feature 63's lo x W_hi
    cross-term is dropped -> ~1e-4 logit error.)
  * Bins padded 101 -> 102: pad logit = -100 (exp ~ 0), and the pad
    column carries out_interp so one DMA writes both outputs.
  * Per 10-tile group (2 PSUM banks): grouped exp on ScalarE, segmented
    softmax sums + interp dot on VectorE, normalize on the otherwise-idle
    GpSimd via a stride-0 broadcast multiply.
  * All DRAM I/O is partition-major ([128, ...] contiguous per
    partition) so every DMA is a 2D transfer with 2.5-4KB bursts; the
    host does the cheap reindexing.
"""

import os
import sys
import types
from contextlib import ExitStack

import numpy as np

import concourse.bass as bass
import concourse.tile as tile
from concourse import bacc, mybir
from concourse import bass_utils

F32 = mybir.dt.float32
F16 = mybir.dt.float16

N_CORES = 8
NBINS = 101
NB_PAD = 102
IND = 64
TILE = 128           # rows per tile (SBUF partitions)
TPB = 5              # tiles per PSUM bank (5 * 102 = 510 <= 512 floats)
BPG = 2              # PSUM banks per group
TPG = TPB * BPG      # tiles per group = 10
GROUP_ROWS = TILE * TPG          # 1280
WINDOW_TILES = TPB               # tiles sharing one gather window
WIN = 4              # gather window width (needs intra-window Li spread <= 2)
PAD_LOGIT = -100.0
BANK = 512           # PSUM bank stride in f32 elements
K1 = 128             # MM1 contraction: ones + x_hi(64) + x_lo(0:63)
K2 = 65              # MM2 contraction: ones + x_hi(64)

LAST_RESULT = None   # stashed for the local test harness


def _install_ntff_hook():
    try:
        from antenv.axon_hooks import get_axon_ntff_profile_hook  # noqa: F401
        return
    except ImportError:
        pass
    try:
        import antenv
        from trn_agent_boot.trn_boot import _ntff_profile_via_ctypes
        mod = types.ModuleType("antenv.axon_hooks")
        hook = [_ntff_profile_via_ctypes("/opt/axon/libaxon_pjrt.so")]
        mod.set_axon_ntff_profile_hook = lambda h: hook.__setitem__(0, h)
        mod.get_axon_ntff_profile_hook = lambda: hook[0]
        sys.modules["antenv.axon_hooks"] = mod
        antenv.axon_hooks = mod
    except Exception:
        pass


def host_prepare(t, x, weight, bias, num_grid, n_cores=N_CORES):
    """Sort/shard/precompute on host.  Returns (meta, per-core input maps)."""
    t = np.ascontiguousarray(np.asarray(t, dtype=np.float32))
    x = np.asarray(x, dtype=np.float32)
    weight = np.asarray(weight, dtype=np.float32)
    bias = np.asarray(bias, dtype=np.float32)
    B = int(num_grid)
    N = t.shape[0]

    # Bin indices / interpolation weight, float32-exact vs the reference.
    tB = t * np.float32(B)
    U = np.ceil(tB)
    inter = np.float32(1.0) - (U - tB)
    L = U - np.float32(1.0)
    L = np.where(L < 0, L + np.float32(1.0), L)
    Li = L.astype(np.int32)
    Ui = U.astype(np.int32)

    chunk = n_cores * GROUP_ROWS
    NP = ((N + chunk - 1) // chunk) * chunk
    npad = NP - N
    R = NP // n_cores
    J = R // TILE                      # tiles per core
    n_windows = J // WINDOW_TILES
    winrows = TILE * n_cores * WINDOW_TILES

    perm = np.argsort(Li, kind="stable")
    Li_s = np.concatenate([Li[perm], np.full(npad, Li[perm[-1]] if N else 0, np.int32)])
    Ui_s = np.concatenate([Ui[perm], np.full(npad, 0, np.int32)])
    inter_s = np.concatenate([inter[perm], np.zeros(npad, np.float32)])

    LO = np.minimum(Li_s[::winrows], NB_PAD - WIN).astype(np.int32)
    assert LO.shape[0] == n_windows
    lo_per_row = np.repeat(LO, winrows)
    spread_ok = (Li_s - lo_per_row >= 0) & (np.maximum(Li_s, Ui_s) - lo_per_row < WIN)
    if not spread_ok[:N].all():
        bad = np.flatnonzero(~spread_ok[:N])[:5]
        raise AssertionError(f"gather-window assumption violated at sorted rows {bad}")

    coef_s = np.zeros((NP, WIN), np.float32)
    rows = np.arange(N)
    np.add.at(coef_s, (rows, (Li_s[:N] - lo_per_row[:N])), np.float32(1.0) - inter_s[:N])
    np.add.at(coef_s, (rows, (Ui_s[:N] - lo_per_row[:N])), inter_s[:N])

    # Weight/bias fp16 hi/lo, padded bins.
    wb = np.zeros((IND + 1, NB_PAD), np.float32)   # row 0..63 = W, row 64 = bias
    wb[:IND, :NBINS] = weight
    wb[IND, :NBINS] = bias
    wb[IND, NBINS] = np.float32(PAD_LOGIT)
    w_hi = wb.astype(np.float16)
    w_lo = (wb - w_hi.astype(np.float32)).astype(np.float16)
    # MM1 rhs [128, 102]: [b_hi; W_hi; W_hi(0:63)]
    w1 = np.empty((K1, NB_PAD), np.float16)
    w1[0] = w_hi[IND]
    w1[1:IND + 1] = w_hi[:IND]
    w1[IND + 1:] = w_hi[:IND - 1]
    # MM2 rhs [65, 102]: [b_lo; W_lo]
    w2 = np.empty((K2, NB_PAD), np.float16)
    w2[0] = w_lo[IND]
    w2[1:] = w_lo[:IND]

    # Sorted+padded x -> per-core fp16 stack [128, R]:
    # partition 0 = ones, 1..64 = x_hi, 65..127 = x_lo(features 0:63).
    xs = np.zeros((NP, IND), np.float32)
    xs[:N] = x[perm]
    in_maps = []
    for i in range(n_cores):
        xi = xs[i::n_cores]                       # [R, 64] f32
        xi_hi = xi.astype(np.float16)
        xi_lo = (xi - xi_hi.astype(np.float32)).astype(np.float16)
        xst = np.empty((TILE, R), np.float16)
        xst[0] = np.float16(1.0)
        xst[1:IND + 1] = xi_hi.T
        xst[IND + 1:] = xi_lo[:, :IND - 1].T
        # coef partition-major: [128, J*4]
        ci = coef_s[i::n_cores].reshape(J, TILE, WIN).transpose(1, 0, 2) \
            .reshape(TILE, J * WIN)
        in_maps.append({
            "xst": xst,
            "w1": w1,
            "w2": w2,
            "coef": np.ascontiguousarray(ci),
        })

    meta = dict(N=N, NP=NP, R=R, J=J, LO=LO, perm=perm, n_cores=n_cores)
    return meta, in_maps


def build_program(LO, R, n_cores=N_CORES):
    """Build + compile the (SPMD-identical) Bass program for one core."""
    J = R // TILE
    n_groups = R // GROUP_ROWS
    assert n_groups * GROUP_ROWS == R
    assert len(LO) == J // WINDOW_TILES

    nc = bacc.Bacc("TRN2", target_bir_lowering=False, debug=False,
                   num_devices=n_cores)
    xst = nc.dram_tensor("xst", [TILE, R], F16, kind="ExternalInput").ap()
    w1 = nc.dram_tensor("w1", [K1, NB_PAD], F16, kind="ExternalInput").ap()
    w2 = nc.dram_tensor("w2", [K2, NB_PAD], F16, kind="ExternalInput").ap()
    coef = nc.dram_tensor("coef", [TILE, J * WIN], F32, kind="ExternalInput").ap()
    # Partition-major combined output: [128, J*102]; per tile j cols
    # j*102 .. j*102+101 = out1 row block, col j*102+101 = out_interp.
    comb = nc.dram_tensor("comb", [TILE, J * NB_PAD], F32, kind="ExternalOutput").ap()

    Exp = mybir.ActivationFunctionType.Exp
    mult = mybir.AluOpType.mult
    add = mybir.AluOpType.add
    X = mybir.AxisListType.X

    with tile.TileContext(nc) as tc:
        with ExitStack() as ctx:
            wpool = ctx.enter_context(tc.tile_pool(name="w", bufs=1))
            xpool = ctx.enter_context(tc.tile_pool(name="x", bufs=4))
            cpool = ctx.enter_context(tc.tile_pool(name="c", bufs=4))
            ppool = ctx.enter_context(tc.tile_pool(name="ps", bufs=3, space="PSUM"))
            epool = ctx.enter_context(tc.tile_pool(name="ex", bufs=3))
            opool = ctx.enter_context(tc.tile_pool(name="o1", bufs=3))
            spool = ctx.enter_context(tc.tile_pool(name="sm", bufs=4))
            tpool = ctx.enter_context(tc.tile_pool(name="tt", bufs=4))

            w1t = wpool.tile([K1, NB_PAD], F16)
            nc.sync.dma_start(w1t[:], w1[:])
            w2t = wpool.tile([K2, NB_PAD], F16)
            nc.sync.dma_start(w2t[:], w2[:])

            for g in range(n_groups):
                c0 = g * GROUP_ROWS          # column offset into xst
                xt = xpool.tile([TILE, GROUP_ROWS], F16)
                nc.scalar.dma_start(xt[:], xst[:, c0:c0 + GROUP_ROWS])
                cf = cpool.tile([TILE, TPG * WIN], F32)
                nc.scalar.dma_start(cf[:], coef[:, g * TPG * WIN:(g + 1) * TPG * WIN])

                ps = ppool.tile([128, BPG * BANK], F32)
                for ti in range(TPG):
                    o = (ti // TPB) * BANK + (ti % TPB) * NB_PAD
                    xsl = slice(ti * TILE, (ti + 1) * TILE)
                    nc.tensor.matmul(ps[:, o:o + NB_PAD], lhsT=xt[:, xsl],
                                     rhs=w1t[:], start=True, stop=False)
                    nc.tensor.matmul(ps[:, o:o + NB_PAD], lhsT=xt[0:K2, xsl],
                                     rhs=w2t[:], start=False, stop=True)

                ex = epool.tile([128, TPG * NB_PAD], F32)
                for b in range(BPG):
                    nc.scalar.activation(
                        ex[:, b * TPB * NB_PAD:(b + 1) * TPB * NB_PAD],
                        ps[:, b * BANK: b * BANK + TPB * NB_PAD],
                        Exp,
                    )

                sg = spool.tile([128, TPG], F32)
                for b in range(BPG):
                    nc.vector.tensor_reduce(
                        sg[:, b * TPB:(b + 1) * TPB],
                        ex[:, b * TPB * NB_PAD:(b + 1) * TPB * NB_PAD]
                          .rearrange("p (t c) -> p t c", t=TPB),
                        axis=X, op=add,
                    )
                rg = spool.tile([128, TPG], F32)
                nc.vector.reciprocal(rg[:], sg[:])

                o1 = opool.tile([128, TPG * NB_PAD], F32)
                nc.gpsimd.tensor_tensor(
                    o1[:].rearrange("p (t c) -> p t c", t=TPG),
                    ex[:].rearrange("p (t c) -> p t c", t=TPG),
                    rg[:].broadcast_to((128, TPG, NB_PAD)),
                    op=mult,
                )

                tt = tpool.tile([128, TPG * WIN], F32)
                for w in range(TPG // WINDOW_TILES):
                    lo = int(LO[(g * TPG) // WINDOW_TILES + w])
                    wsl = slice(w * WINDOW_TILES, (w + 1) * WINDOW_TILES)
                    nc.vector.tensor_tensor(
                        tt[:, w * WINDOW_TILES * WIN:(w + 1) * WINDOW_TILES * WIN]
                          .rearrange("p (t c) -> p t c", t=WINDOW_TILES),
                        o1[:].rearrange("p (t c) -> p t c", t=TPG)[:, wsl, lo:lo + WIN],
                        cf[:].rearrange("p (t c) -> p t c", t=TPG)[:, wsl, :],
                        op=mult,
                    )
                    nc.vector.tensor_reduce(
                        o1[:].rearrange("p (t c) -> p t c", t=TPG)[:, wsl, NBINS:NB_PAD],
                        tt[:, w * WINDOW_TILES * WIN:(w + 1) * WINDOW_TILES * WIN]
                          .rearrange("p (t c) -> p t c", t=WINDOW_TILES),
                        axis=X, op=add,
                    )

                nc.sync.dma_start(
                    comb[:, g * TPG * NB_PAD:(g + 1) * TPG * NB_PAD], o1[:])

    nc.compile()
    return nc


def kernel(t, x, weight, bias, num_grid):
    global LAST_RESULT
    trace = bool(os.environ.get("BASS_TRACE"))
    if trace:
        _install_ntff_hook()
        bass_utils.upload_artifacts = lambda tmpdir: "local://" + tmpdir

    meta, in_maps = host_prepare(t, x, weight, bias, num_grid)
    nc = build_program(meta["LO"], meta["R"], meta["n_cores"])

    res = bass_utils.run_bass_kernel_spmd(
        nc, in_maps, core_ids=list(range(meta["n_cores"])), trace=trace,
    )
    LAST_RESULT = res

    N, NP, n_cores = meta["N"], meta["NP"], meta["n_cores"]
    R, J = meta["R"], meta["J"]
    perm = meta["perm"]
    comb_s = np.empty((NP, NB_PAD), np.float32)
    for i in range(n_cores):
        ci = res.results[i]["comb"].reshape(TILE, J, NB_PAD)
        comb_s[i::n_cores] = ci.transpose(1, 0, 2).reshape(R, NB_PAD)
    out1 = np.empty((N, NBINS), np.float32)
    oint = np.empty((N,), np.float32)
    out1[perm] = comb_s[:N, :NBINS]
    oint[perm] = comb_s[:N, NBINS]
    return out1, oint


# revision 10
# speedup vs baseline: 8.0840x; 1.2584x over previous
"""Trainium2 Bass kernel for nn_Density_Block (histogram_binning).

Computes, for N=1M rows:
    out1       = softmax(x @ weight + bias, axis=1)        [N, 101]
    out_interp = lerp of two adjacent bins of out1 at t*B   [N]

Strategy (8 NeuronCores, pure data parallel):
  * Host sorts rows by lower-bin index Li and interleaves sorted rows
    round-robin across cores, so every run of 5 consecutive 128-row tiles
    (on every core) gathers its two softmax bins from one 4-wide column
    window known at program-build time.
  * The matmul x@W+b runs as TWO fp16 matmuls accumulated in fp32 PSUM:
      MM1 (K=128): [ones; x_hi; x_lo(0:63)] @ [b_hi; W_hi; W_hi(0:63)]
      MM2 (K=65):  [ones; x_hi]             @ [b_lo; W_lo]
    (x_hi/W_hi fp16, *_lo = fp16 residuals; only feature 63's lo x W_hi
    cross-term is dropped -> ~1e-4 logit error.)
  * Bins padded 101 -> 102: pad logit = -100 (exp ~ 0), and the pad
    column carries out_interp so one DMA writes both outputs.
  * Per 10-tile group (2 PSUM banks): grouped exp on ScalarE, segmented
    softmax sums + interp dot on VectorE, normalize on the otherwise-idle
    GpSimd via a stride-0 broadcast multiply.
  * All DRAM I/O is partition-major ([128, ...] contiguous per
    partition) so every DMA is a 2D transfer with 2.5-4KB bursts; the
    host does the cheap reindexing.
"""

import os
import sys
import types
from contextlib import ExitStack

import numpy as np

import concourse.bass as bass
import concourse.tile as tile
from concourse import bacc, mybir
from concourse import bass_utils

F32 = mybir.dt.float32
F16 = mybir.dt.float16

N_CORES = 8
NBINS = 101
NB_PAD = 102
IND = 64
TILE = 128           # rows per tile (SBUF partitions)
TPB = 5              # tiles per PSUM bank (5 * 102 = 510 <= 512 floats)
BPG = 2              # PSUM banks per group
TPG = TPB * BPG      # tiles per group = 10
GROUP_ROWS = TILE * TPG          # 1280
WINDOW_TILES = TPB               # tiles sharing one gather window
WIN = 4              # gather window width (needs intra-window Li spread <= 2)
PAD_LOGIT = -100.0
BANK = 512           # PSUM bank stride in f32 elements
K1 = 128             # MM1 contraction: ones + x_hi(64) + x_lo(0:63)
K2 = 65              # MM2 contraction: ones + x_hi(64)

LAST_RESULT = None   # stashed for the local test harness


def _install_ntff_hook():
    try:
        from antenv.axon_hooks import get_axon_ntff_profile_hook  # noqa: F401
        return
    except ImportError:
        pass
    try:
        import antenv
        from trn_agent_boot.trn_boot import _ntff_profile_via_ctypes
        mod = types.ModuleType("antenv.axon_hooks")
        hook = [_ntff_profile_via_ctypes("/opt/axon/libaxon_pjrt.so")]
        mod.set_axon_ntff_profile_hook = lambda h: hook.__setitem__(0, h)
        mod.get_axon_ntff_profile_hook = lambda: hook[0]
        sys.modules["antenv.axon_hooks"] = mod
        antenv.axon_hooks = mod
    except Exception:
        pass


def host_prepare(t, x, weight, bias, num_grid, n_cores=N_CORES):
    """Sort/shard/precompute on host.  Returns (meta, per-core input maps)."""
    t = np.ascontiguousarray(np.asarray(t, dtype=np.float32))
    x = np.asarray(x, dtype=np.float32)
    weight = np.asarray(weight, dtype=np.float32)
    bias = np.asarray(bias, dtype=np.float32)
    B = int(num_grid)
    N = t.shape[0]

    # Bin indices / interpolation weight, float32-exact vs the reference.
    tB = t * np.float32(B)
    U = np.ceil(tB)
    inter = np.float32(1.0) - (U - tB)
    L = U - np.float32(1.0)
    L = np.where(L < 0, L + np.float32(1.0), L)
    Li = L.astype(np.int32)
    Ui = U.astype(np.int32)

    chunk = n_cores * GROUP_ROWS
    NP = ((N + chunk - 1) // chunk) * chunk
    npad = NP - N
    R = NP // n_cores
    J = R // TILE                      # tiles per core
    n_windows = J // WINDOW_TILES
    winrows = TILE * n_cores * WINDOW_TILES

    perm = np.argsort(Li, kind="stable")
    Li_s = np.concatenate([Li[perm], np.full(npad, Li[perm[-1]] if N else 0, np.int32)])
    Ui_s = np.concatenate([Ui[perm], np.full(npad, 0, np.int32)])
    inter_s = np.concatenate([inter[perm], np.zeros(npad, np.float32)])

    LO = np.minimum(Li_s[::winrows], NB_PAD - WIN).astype(np.int32)
    assert LO.shape[0] == n_windows
    lo_per_row = np.repeat(LO, winrows)
    spread_ok = (Li_s - lo_per_row >= 0) & (np.maximum(Li_s, Ui_s) - lo_per_row < WIN)
    if not spread_ok[:N].all():
        bad = np.flatnonzero(~spread_ok[:N])[:5]
        raise AssertionError(f"gather-window assumption violated at sorted rows {bad}")

    coef_s = np.zeros((NP, WIN), np.float32)
    rows = np.arange(N)
    np.add.at(coef_s, (rows, (Li_s[:N] - lo_per_row[:N])), np.float32(1.0) - inter_s[:N])
    np.add.at(coef_s, (rows, (Ui_s[:N] - lo_per_row[:N])), inter_s[:N])

    # Weight/bias fp16 hi/lo, padded bins.
    wb = np.zeros((IND + 1, NB_PAD), np.float32)   # row 0..63 = W, row 64 = bias
    wb[:IND, :NBINS] = weight
    wb[IND, :NBINS] = bias
    wb[IND, NBINS] = np.float32(PAD_LOGIT)
    w_hi = wb.astype(np.float16)
    w_lo = (wb - w_hi.astype(np.float32)).astype(np.float16)
    # MM1 rhs [128, 102]: [b_hi; W_hi; W_hi(0:63)]
    w1 = np.empty((K1, NB_PAD), np.float16)
    w1[0] = w_hi[IND]
    w1[1:IND + 1] = w_hi[:IND]
    w1[IND + 1:] = w_hi[:IND - 1]
    # MM2 rhs [65, 102]: [b_lo; W_lo]
    w2 = np.empty((K2, NB_PAD), np.float16)
    w2[0] = w_lo[IND]
    w2[1:] = w_lo[:IND]

    # Sorted+padded x -> per-core fp16 stack [128, R]:
    # partition 0 = ones, 1..64 = x_hi, 65..127 = x_lo(features 0:63).
    xs = np.zeros((NP, IND), np.float32)
    xs[:N] = x[perm]
    in_maps = []
    for i in range(n_cores):
        xi = xs[i::n_cores]                       # [R, 64] f32
        xi_hi = xi.astype(np.float16)
        xi_lo = (xi - xi_hi.astype(np.float32)).astype(np.float16)
        xst = np.empty((TILE, R), np.float16)
        xst[0] = np.float16(1.0)
        xst[1:IND + 1] = xi_hi.T
        xst[IND + 1:] = xi_lo[:, :IND - 1].T
        # coef partition-major: [128, J*4]
        ci = coef_s[i::n_cores].reshape(J, TILE, WIN).transpose(1, 0, 2) \
            .reshape(TILE, J * WIN)
        in_maps.append({
            "xst": xst,
            "w1": w1,
            "w2": w2,
            "coef": np.ascontiguousarray(ci),
        })

    meta = dict(N=N, NP=NP, R=R, J=J, LO=LO, perm=perm, n_cores=n_cores)
    return meta, in_maps


def build_program(LO, R, n_cores=N_CORES):
    """Build + compile the (SPMD-identical) Bass program for one core."""
    J = R // TILE
    n_groups = R // GROUP_ROWS
    assert n_groups * GROUP_ROWS == R
    assert len(LO) == J // WINDOW_TILES

    nc = bacc.Bacc("TRN2", target_bir_lowering=False, debug=False,
                   num_devices=n_cores)
    xst = nc.dram_tensor("xst", [TILE, R], F16, kind="ExternalInput").ap()
    w1 = nc.dram_tensor("w1", [K1, NB_PAD], F16, kind="ExternalInput").ap()
    w2 = nc.dram_tensor("w2", [K2, NB_PAD], F16, kind="ExternalInput").ap()
    coef = nc.dram_tensor("coef", [TILE, J * WIN], F32, kind="ExternalInput").ap()
    # Partition-major combined output: [128, J*102]; per tile j cols
    # j*102 .. j*102+101 = out1 row block, col j*102+101 = out_interp.
    comb = nc.dram_tensor("comb", [TILE, J * NB_PAD], F32, kind="ExternalOutput").ap()

    Exp = mybir.ActivationFunctionType.Exp
    mult = mybir.AluOpType.mult
    add = mybir.AluOpType.add
    X = mybir.AxisListType.X

    with tile.TileContext(nc) as tc:
        with ExitStack() as ctx:
            wpool = ctx.enter_context(tc.tile_pool(name="w", bufs=1))
            xpool = ctx.enter_context(tc.tile_pool(name="x", bufs=6))
            cpool = ctx.enter_context(tc.tile_pool(name="c", bufs=6))
            ppool = ctx.enter_context(tc.tile_pool(name="ps", bufs=4, space="PSUM"))
            epool = ctx.enter_context(tc.tile_pool(name="ex", bufs=6))
            opool = ctx.enter_context(tc.tile_pool(name="o1", bufs=6))
            spool = ctx.enter_context(tc.tile_pool(name="sm", bufs=8))
            tpool = ctx.enter_context(tc.tile_pool(name="tt", bufs=8))

            w1t = wpool.tile([K1, NB_PAD], F16)
            nc.sync.dma_start(w1t[:], w1[:])
            w2t = wpool.tile([K2, NB_PAD], F16)
            nc.sync.dma_start(w2t[:], w2[:])

            for g in range(n_groups):
                c0 = g * GROUP_ROWS          # column offset into xst
                xt = xpool.tile([TILE, GROUP_ROWS], F16)
                nc.scalar.dma_start(xt[:], xst[:, c0:c0 + GROUP_ROWS])
                cf = cpool.tile([TILE, TPG * WIN], F32)
                nc.sync.dma_start(cf[:], coef[:, g * TPG * WIN:(g + 1) * TPG * WIN])

                ps = ppool.tile([128, BPG * BANK], F32)
                for ti in range(TPG):
                    o = (ti // TPB) * BANK + (ti % TPB) * NB_PAD
                    xsl = slice(ti * TILE, (ti + 1) * TILE)
                    nc.tensor.matmul(ps[:, o:o + NB_PAD], lhsT=xt[:, xsl],
                                     rhs=w1t[:], start=True, stop=False)
                    nc.tensor.matmul(ps[:, o:o + NB_PAD], lhsT=xt[0:K2, xsl],
                                     rhs=w2t[:], start=False, stop=True)

                ex = epool.tile([128, TPG * NB_PAD], F32)
                nc.scalar.activation(
                    ex[:].rearrange("p (b c) -> p b c", b=BPG),
                    ps[:].rearrange("p (b c) -> p b c", b=BPG)[:, :, 0:TPB * NB_PAD],
                    Exp,
                )

                sg = spool.tile([128, TPG], F32)
                nc.vector.tensor_reduce(
                    sg[:],
                    ex[:].rearrange("p (t c) -> p t c", t=TPG),
                    axis=X, op=add,
                )
                rg = spool.tile([128, TPG], F32)
                nc.vector.reciprocal(rg[:], sg[:])

                o1 = opool.tile([128, TPG * NB_PAD], F32)
                nc.gpsimd.tensor_tensor(
                    o1[:].rearrange("p (t c) -> p t c", t=TPG),
                    ex[:].rearrange("p (t c) -> p t c", t=TPG),
                    rg[:].broadcast_to((128, TPG, NB_PAD)),
                    op=mult,
                )

                tt = tpool.tile([128, TPG * WIN], F32)
                for w in range(TPG // WINDOW_TILES):
                    lo = int(LO[(g * TPG) // WINDOW_TILES + w])
                    wsl = slice(w * WINDOW_TILES, (w + 1) * WINDOW_TILES)
                    nc.vector.tensor_tensor(
                        tt[:, w * WINDOW_TILES * WIN:(w + 1) * WINDOW_TILES * WIN]
                          .rearrange("p (t c) -> p t c", t=WINDOW_TILES),
                        o1[:].rearrange("p (t c) -> p t c", t=TPG)[:, wsl, lo:lo + WIN],
                        cf[:].rearrange("p (t c) -> p t c", t=TPG)[:, wsl, :],
                        op=mult,
                    )
                    nc.vector.tensor_reduce(
                        o1[:].rearrange("p (t c) -> p t c", t=TPG)[:, wsl, NBINS:NB_PAD],
                        tt[:, w * WINDOW_TILES * WIN:(w + 1) * WINDOW_TILES * WIN]
                          .rearrange("p (t c) -> p t c", t=WINDOW_TILES),
                        axis=X, op=add,
                    )

                nc.sync.dma_start(
                    comb[:, g * TPG * NB_PAD:(g + 1) * TPG * NB_PAD], o1[:])

    nc.compile()
    return nc


def kernel(t, x, weight, bias, num_grid):
    global LAST_RESULT
    trace = bool(os.environ.get("BASS_TRACE"))
    if trace:
        _install_ntff_hook()
        bass_utils.upload_artifacts = lambda tmpdir: "local://" + tmpdir

    meta, in_maps = host_prepare(t, x, weight, bias, num_grid)
    nc = build_program(meta["LO"], meta["R"], meta["n_cores"])

    res = bass_utils.run_bass_kernel_spmd(
        nc, in_maps, core_ids=list(range(meta["n_cores"])), trace=trace,
    )
    LAST_RESULT = res

    N, NP, n_cores = meta["N"], meta["NP"], meta["n_cores"]
    R, J = meta["R"], meta["J"]
    perm = meta["perm"]
    comb_s = np.empty((NP, NB_PAD), np.float32)
    for i in range(n_cores):
        ci = res.results[i]["comb"].reshape(TILE, J, NB_PAD)
        comb_s[i::n_cores] = ci.transpose(1, 0, 2).reshape(R, NB_PAD)
    out1 = np.empty((N, NBINS), np.float32)
    oint = np.empty((N,), np.float32)
    out1[perm] = comb_s[:N, :NBINS]
    oint[perm] = comb_s[:N, NBINS]
    return out1, oint


# revision 11
# speedup vs baseline: 8.1041x; 1.0025x over previous
"""Trainium2 Bass kernel for nn_Density_Block (histogram_binning).

Computes, for N=1M rows:
    out1       = softmax(x @ weight + bias, axis=1)        [N, 101]
    out_interp = lerp of two adjacent bins of out1 at t*B   [N]

Strategy (8 NeuronCores, pure data parallel):
  * Host sorts rows by lower-bin index Li and interleaves sorted rows
    round-robin across cores, so every run of 5 consecutive 128-row tiles
    (on every core) gathers its two softmax bins from one 4-wide column
    window known at program-build time.
  * The matmul x@W+b runs as TWO fp16 matmuls accumulated in fp32 PSUM:
      MM1 (K=128): [ones; x_hi; x_lo(0:63)] @ [b_hi; W_hi; W_hi(0:63)]
      MM2 (K=65):  [ones; x_hi]             @ [b_lo; W_lo]
    (x_hi/W_hi fp16, *_lo = fp16 residuals; only feature 63's lo x W_hi
    cross-term is dropped -> ~1e-4 logit error.)
  * Bins padded 101 -> 102: pad logit = -100 (exp ~ 0), and the pad
    column carries out_interp so one DMA writes both outputs.
  * Per 10-tile group (2 PSUM banks): grouped exp on ScalarE, segmented
    softmax sums + interp dot on VectorE, normalize on the otherwise-idle
    GpSimd via a stride-0 broadcast multiply.
  * All DRAM I/O is partition-major ([128, ...] contiguous per
    partition) so every DMA is a 2D transfer with 2.5-4KB bursts; the
    host does the cheap reindexing.
"""

import os
import sys
import types
from contextlib import ExitStack

import numpy as np

import concourse.bass as bass
import concourse.tile as tile
from concourse import bacc, mybir
from concourse import bass_utils

F32 = mybir.dt.float32
F16 = mybir.dt.float16

N_CORES = 8
NBINS = 101
NB_PAD = 102
IND = 64
TILE = 128           # rows per tile (SBUF partitions)
TPB = 5              # tiles per PSUM bank (5 * 102 = 510 <= 512 floats)
BPG = 2              # PSUM banks per group
TPG = TPB * BPG      # tiles per group = 10
GROUP_ROWS = TILE * TPG          # 1280
WINDOW_TILES = TPG               # tiles sharing one gather window (whole group)
WIN = 4              # gather window width (needs intra-window Li spread <= 2)
PAD_LOGIT = -100.0
BANK = 512           # PSUM bank stride in f32 elements
K1 = 128             # MM1 contraction: ones + x_hi(64) + x_lo(0:63)
K2 = 65              # MM2 contraction: ones + x_hi(64)

LAST_RESULT = None   # stashed for the local test harness


def _install_ntff_hook():
    try:
        from antenv.axon_hooks import get_axon_ntff_profile_hook  # noqa: F401
        return
    except ImportError:
        pass
    try:
        import antenv
        from trn_agent_boot.trn_boot import _ntff_profile_via_ctypes
        mod = types.ModuleType("antenv.axon_hooks")
        hook = [_ntff_profile_via_ctypes("/opt/axon/libaxon_pjrt.so")]
        mod.set_axon_ntff_profile_hook = lambda h: hook.__setitem__(0, h)
        mod.get_axon_ntff_profile_hook = lambda: hook[0]
        sys.modules["antenv.axon_hooks"] = mod
        antenv.axon_hooks = mod
    except Exception:
        pass


def host_prepare(t, x, weight, bias, num_grid, n_cores=N_CORES):
    """Sort/shard/precompute on host.  Returns (meta, per-core input maps)."""
    t = np.ascontiguousarray(np.asarray(t, dtype=np.float32))
    x = np.asarray(x, dtype=np.float32)
    weight = np.asarray(weight, dtype=np.float32)
    bias = np.asarray(bias, dtype=np.float32)
    B = int(num_grid)
    N = t.shape[0]

    # Bin indices / interpolation weight, float32-exact vs the reference.
    tB = t * np.float32(B)
    U = np.ceil(tB)
    inter = np.float32(1.0) - (U - tB)
    L = U - np.float32(1.0)
    L = np.where(L < 0, L + np.float32(1.0), L)
    Li = L.astype(np.int32)
    Ui = U.astype(np.int32)

    chunk = n_cores * GROUP_ROWS
    NP = ((N + chunk - 1) // chunk) * chunk
    npad = NP - N
    R = NP // n_cores
    J = R // TILE                      # tiles per core
    n_windows = J // WINDOW_TILES
    winrows = TILE * n_cores * WINDOW_TILES

    perm = np.argsort(Li, kind="stable")
    Li_s = np.concatenate([Li[perm], np.full(npad, Li[perm[-1]] if N else 0, np.int32)])
    Ui_s = np.concatenate([Ui[perm], np.full(npad, 0, np.int32)])
    inter_s = np.concatenate([inter[perm], np.zeros(npad, np.float32)])

    LO = np.minimum(Li_s[::winrows], NB_PAD - WIN).astype(np.int32)
    assert LO.shape[0] == n_windows
    lo_per_row = np.repeat(LO, winrows)
    spread_ok = (Li_s - lo_per_row >= 0) & (np.maximum(Li_s, Ui_s) - lo_per_row < WIN)
    if not spread_ok[:N].all():
        bad = np.flatnonzero(~spread_ok[:N])[:5]
        raise AssertionError(f"gather-window assumption violated at sorted rows {bad}")

    coef_s = np.zeros((NP, WIN), np.float32)
    rows = np.arange(N)
    np.add.at(coef_s, (rows, (Li_s[:N] - lo_per_row[:N])), np.float32(1.0) - inter_s[:N])
    np.add.at(coef_s, (rows, (Ui_s[:N] - lo_per_row[:N])), inter_s[:N])

    # Weight/bias fp16 hi/lo, padded bins.
    wb = np.zeros((IND + 1, NB_PAD), np.float32)   # row 0..63 = W, row 64 = bias
    wb[:IND, :NBINS] = weight
    wb[IND, :NBINS] = bias
    wb[IND, NBINS] = np.float32(PAD_LOGIT)
    w_hi = wb.astype(np.float16)
    w_lo = (wb - w_hi.astype(np.float32)).astype(np.float16)
    # MM1 rhs [128, 102]: [b_hi; W_hi; W_hi(0:63)]
    w1 = np.empty((K1, NB_PAD), np.float16)
    w1[0] = w_hi[IND]
    w1[1:IND + 1] = w_hi[:IND]
    w1[IND + 1:] = w_hi[:IND - 1]
    # MM2 rhs [65, 102]: [b_lo; W_lo]
    w2 = np.empty((K2, NB_PAD), np.float16)
    w2[0] = w_lo[IND]
    w2[1:] = w_lo[:IND]

    # Sorted+padded x -> per-core fp16 stack [128, R]:
    # partition 0 = ones, 1..64 = x_hi, 65..127 = x_lo(features 0:63).
    xs = np.zeros((NP, IND), np.float32)
    xs[:N] = x[perm]
    in_maps = []
    for i in range(n_cores):
        xi = xs[i::n_cores]                       # [R, 64] f32
        xi_hi = xi.astype(np.float16)
        xi_lo = (xi - xi_hi.astype(np.float32)).astype(np.float16)
        xst = np.empty((TILE, R), np.float16)
        xst[0] = np.float16(1.0)
        xst[1:IND + 1] = xi_hi.T
        xst[IND + 1:] = xi_lo[:, :IND - 1].T
        # coef partition-major: [128, J*4]
        ci = coef_s[i::n_cores].reshape(J, TILE, WIN).transpose(1, 0, 2) \
            .reshape(TILE, J * WIN)
        in_maps.append({
            "xst": xst,
            "w1": w1,
            "w2": w2,
            "coef": np.ascontiguousarray(ci),
        })

    meta = dict(N=N, NP=NP, R=R, J=J, LO=LO, perm=perm, n_cores=n_cores)
    return meta, in_maps


def build_program(LO, R, n_cores=N_CORES):
    """Build + compile the (SPMD-identical) Bass program for one core."""
    J = R // TILE
    n_groups = R // GROUP_ROWS
    assert n_groups * GROUP_ROWS == R
    assert len(LO) == J // WINDOW_TILES

    nc = bacc.Bacc("TRN2", target_bir_lowering=False, debug=False,
                   num_devices=n_cores)
    xst = nc.dram_tensor("xst", [TILE, R], F16, kind="ExternalInput").ap()
    w1 = nc.dram_tensor("w1", [K1, NB_PAD], F16, kind="ExternalInput").ap()
    w2 = nc.dram_tensor("w2", [K2, NB_PAD], F16, kind="ExternalInput").ap()
    coef = nc.dram_tensor("coef", [TILE, J * WIN], F32, kind="ExternalInput").ap()
    # Partition-major combined output: [128, J*102]; per tile j cols
    # j*102 .. j*102+101 = out1 row block, col j*102+101 = out_interp.
    comb = nc.dram_tensor("comb", [TILE, J * NB_PAD], F32, kind="ExternalOutput").ap()

    Exp = mybir.ActivationFunctionType.Exp
    mult = mybir.AluOpType.mult
    add = mybir.AluOpType.add
    X = mybir.AxisListType.X

    with tile.TileContext(nc) as tc:
        with ExitStack() as ctx:
            wpool = ctx.enter_context(tc.tile_pool(name="w", bufs=1))
            xpool = ctx.enter_context(tc.tile_pool(name="x", bufs=6))
            cpool = ctx.enter_context(tc.tile_pool(name="c", bufs=6))
            ppool = ctx.enter_context(tc.tile_pool(name="ps", bufs=4, space="PSUM"))
            epool = ctx.enter_context(tc.tile_pool(name="ex", bufs=6))
            opool = ctx.enter_context(tc.tile_pool(name="o1", bufs=6))
            spool = ctx.enter_context(tc.tile_pool(name="sm", bufs=8))
            tpool = ctx.enter_context(tc.tile_pool(name="tt", bufs=8))

            w1t = wpool.tile([K1, NB_PAD], F16)
            nc.sync.dma_start(w1t[:], w1[:])
            w2t = wpool.tile([K2, NB_PAD], F16)
            nc.sync.dma_start(w2t[:], w2[:])

            for g in range(n_groups):
                c0 = g * GROUP_ROWS          # column offset into xst
                xt = xpool.tile([TILE, GROUP_ROWS], F16)
                nc.scalar.dma_start(xt[:], xst[:, c0:c0 + GROUP_ROWS])
                cf = cpool.tile([TILE, TPG * WIN], F32)
                nc.sync.dma_start(cf[:], coef[:, g * TPG * WIN:(g + 1) * TPG * WIN])

                ps = ppool.tile([128, BPG * BANK], F32)
                for ti in range(TPG):
                    o = (ti // TPB) * BANK + (ti % TPB) * NB_PAD
                    xsl = slice(ti * TILE, (ti + 1) * TILE)
                    nc.tensor.matmul(ps[:, o:o + NB_PAD], lhsT=xt[:, xsl],
                                     rhs=w1t[:], start=True, stop=False)
                    nc.tensor.matmul(ps[:, o:o + NB_PAD], lhsT=xt[0:K2, xsl],
                                     rhs=w2t[:], start=False, stop=True)

                ex = epool.tile([128, TPG * NB_PAD], F32)
                nc.scalar.activation(
                    ex[:].rearrange("p (b c) -> p b c", b=BPG),
                    ps[:].rearrange("p (b c) -> p b c", b=BPG)[:, :, 0:TPB * NB_PAD],
                    Exp,
                )

                sg = spool.tile([128, TPG], F32)
                nc.vector.tensor_reduce(
                    sg[:],
                    ex[:].rearrange("p (t c) -> p t c", t=TPG),
                    axis=X, op=add,
                )
                rg = spool.tile([128, TPG], F32)
                nc.vector.reciprocal(rg[:], sg[:])

                o1 = opool.tile([128, TPG * NB_PAD], F32)
                nc.gpsimd.tensor_tensor(
                    o1[:].rearrange("p (t c) -> p t c", t=TPG),
                    ex[:].rearrange("p (t c) -> p t c", t=TPG),
                    rg[:].broadcast_to((128, TPG, NB_PAD)),
                    op=mult,
                )

                tt = tpool.tile([128, TPG * WIN], F32)
                for w in range(TPG // WINDOW_TILES):
                    lo = int(LO[(g * TPG) // WINDOW_TILES + w])
                    wsl = slice(w * WINDOW_TILES, (w + 1) * WINDOW_TILES)
                    nc.vector.tensor_tensor(
                        tt[:, w * WINDOW_TILES * WIN:(w + 1) * WINDOW_TILES * WIN]
                          .rearrange("p (t c) -> p t c", t=WINDOW_TILES),
                        o1[:].rearrange("p (t c) -> p t c", t=TPG)[:, wsl, lo:lo + WIN],
                        cf[:].rearrange("p (t c) -> p t c", t=TPG)[:, wsl, :],
                        op=mult,
                    )
                    nc.vector.tensor_reduce(
                        o1[:].rearrange("p (t c) -> p t c", t=TPG)[:, wsl, NBINS:NB_PAD],
                        tt[:, w * WINDOW_TILES * WIN:(w + 1) * WINDOW_TILES * WIN]
                          .rearrange("p (t c) -> p t c", t=WINDOW_TILES),
                        axis=X, op=add,
                    )

                nc.sync.dma_start(
                    comb[:, g * TPG * NB_PAD:(g + 1) * TPG * NB_PAD], o1[:])

    nc.compile()
    return nc


def kernel(t, x, weight, bias, num_grid):
    global LAST_RESULT
    trace = bool(os.environ.get("BASS_TRACE"))
    if trace:
        _install_ntff_hook()
        bass_utils.upload_artifacts = lambda tmpdir: "local://" + tmpdir

    meta, in_maps = host_prepare(t, x, weight, bias, num_grid)
    nc = build_program(meta["LO"], meta["R"], meta["n_cores"])

    res = bass_utils.run_bass_kernel_spmd(
        nc, in_maps, core_ids=list(range(meta["n_cores"])), trace=trace,
    )
    LAST_RESULT = res

    N, NP, n_cores = meta["N"], meta["NP"], meta["n_cores"]
    R, J = meta["R"], meta["J"]
    perm = meta["perm"]
    comb_s = np.empty((NP, NB_PAD), np.float32)
    for i in range(n_cores):
        ci = res.results[i]["comb"].reshape(TILE, J, NB_PAD)
        comb_s[i::n_cores] = ci.transpose(1, 0, 2).reshape(R, NB_PAD)
    out1 = np.empty((N, NBINS), np.float32)
    oint = np.empty((N,), np.float32)
    out1[perm] = comb_s[:N, :NBINS]
    oint[perm] = comb_s[:N, NBINS]
    return out1, oint
